# revision 3
# baseline (speedup 1.0000x reference)
"""Trainium2 Bass kernel for nn_MultiHeadAttention_85229331022244.

Computation (per batch b):
  xh = x.reshape(B,T,64,16); q/k/v = per-head 64x64 projections of xh
  q,k: interleaved RoPE over the FULL 1024-dim feature axis
  scores = q @ k.T / sqrt(1024)  (single attention map over full D)
  causal softmax; y = attn @ v

Sharding: core i -> batch i//2, q-tile parity i%2 (even/odd 128-row q-tiles
interleaved between the two cores of a batch).  Every core runs an identical
program; causality differences are carried in per-core mask data.

Device layout trick: heads are reordered even-first and paired so the
projections become 8 block-diagonal 128x128 matmuls that produce K^T/Q^T
directly in [feature-on-partition, token] layout, with RoPE partner features
living in chunk c and c+4 at the same partition index.
"""

import math
from contextlib import ExitStack

import numpy as np
import ml_dtypes

import concourse.bass as bass
import concourse.mybir as mybir
import concourse.tile as tile
from concourse import bacc
from concourse.bass import ts, ds
from concourse.masks import make_identity

BF16 = ml_dtypes.bfloat16

D_MODEL = 1024
N_HEADS = 16
HEAD_D = 64
ROPE_BASE = 10000.0
GAMMA = 1.0 / math.sqrt(D_MODEL)

# head pairs per 128-row chunk; chunks 0-3 = even heads, 4-7 = odd heads
HEAD_PAIRS = [(0, 2), (4, 6), (8, 10), (12, 14), (1, 3), (5, 7), (9, 11), (13, 15)]


def _feature_perm():
    """perm[c*128 + p] = original feature index for kernel row (c, p)."""
    perm = np.zeros(1024, dtype=np.int64)
    for c, (ha, hb) in enumerate(HEAD_PAIRS):
        for p in range(128):
            h = ha if p < 64 else hb
            perm[c * 128 + p] = (p % 64) * 16 + h
    return perm


PERM = _feature_perm()
INV_PERM = np.argsort(PERM)


def _block_weights(w):
    """w: (64, 64, 16) -> (8, 128, 128) block-diag per chunk, bf16."""
    out = np.zeros((8, 128, 128), dtype=np.float32)
    for c, (ha, hb) in enumerate(HEAD_PAIRS):
        out[c, :64, :64] = w[:, :, ha]
        out[c, 64:, 64:] = w[:, :, hb]
    return out.astype(BF16)


def _rope_tables(T):
    """cos/sin tables [4, 128, T] bf16 for chunks 0-3 (and partners 4-7)."""
    p = np.arange(128)
    cos = np.zeros((4, 128, T), dtype=np.float32)
    sin = np.zeros((4, 128, T), dtype=np.float32)
    t = np.arange(T, dtype=np.float64)
    for c in range(4):
        f = (p % 64) * 8 + (2 * c + p // 64)  # [128]
        inv_freq = ROPE_BASE ** (-f.astype(np.float64) / 512.0)  # [128]
        ang = inv_freq[:, None] * t[None, :]  # [128, T]
        cos[c] = np.cos(ang).astype(np.float32)
        sin[c] = np.sin(ang).astype(np.float32)
    return cos.astype(BF16), sin.astype(BF16)


def _n_stripes(j):
    return (2 * j + 2 + 3) // 4


def _last_width(j):
    nblk = 2 * j + 2
    w = nblk - 4 * (_n_stripes(j) - 1)
    return w * 128  # 256 (j even) or 512 (j odd)


def _masks_for_core(q_tiles, NQ):
    """[NQ, 128, 512] fp32 additive masks for each q-tile's last stripe."""
    m = np.zeros((NQ, 128, 512), dtype=np.float32)
    for j, G in enumerate(q_tiles):
        k0 = 4 * (_n_stripes(j) - 1) * 128  # global key col of stripe start
        tq = G * 128 + np.arange(128)[:, None]
        tk = k0 + np.arange(512)[None, :]
        m[j] = np.where(tk <= tq, 0.0, -1e9)
    return m


def build_nc(T, NQ):
    """Build the (identical-on-all-cores) Bass program.

    T:  total key length (keys 0..T-1 resident per core)
    NQ: number of 128-row query tiles handled by this core
    Requires: max blocks = 2*(NQ-1)+2 <= T//128, T % 512 == 0, NQ % 4 == 0.
    """
    assert T % 512 == 0 and NQ % 4 == 0
    assert 2 * NQ <= T // 128
    n_kv_stripes = T // 512
    n_q_stripes = NQ * 128 // 512
    dt = mybir.dt

    nc = bacc.Bacc("TRN2", target_bir_lowering=False)
    xpT = nc.dram_tensor("xpT", [8, 128, T], dt.bfloat16, kind="ExternalInput")
    xqT = nc.dram_tensor("xqT", [8, 128, NQ * 128], dt.bfloat16, kind="ExternalInput")
    w2q = nc.dram_tensor("w2q", [8, 128, 128], dt.bfloat16, kind="ExternalInput")
    w2k = nc.dram_tensor("w2k", [8, 128, 128], dt.bfloat16, kind="ExternalInput")
    w2v = nc.dram_tensor("w2v", [8, 128, 128], dt.bfloat16, kind="ExternalInput")
    cosk = nc.dram_tensor("cosk", [4, 128, T], dt.bfloat16, kind="ExternalInput")
    sink = nc.dram_tensor("sink", [4, 128, T], dt.bfloat16, kind="ExternalInput")
    cosq = nc.dram_tensor("cosq", [4, 128, NQ * 128], dt.bfloat16, kind="ExternalInput")
    sinq = nc.dram_tensor("sinq", [4, 128, NQ * 128], dt.bfloat16, kind="ExternalInput")
    masks = nc.dram_tensor("masks", [NQ, 128, 512], dt.float32, kind="ExternalInput")
    y = nc.dram_tensor("y", [NQ * 128, 1024], dt.float32, kind="ExternalOutput")

    with tile.TileContext(nc) as tc, ExitStack() as ctx:
        const = ctx.enter_context(tc.tile_pool(name="const", bufs=1))
        kv = ctx.enter_context(tc.tile_pool(name="kv", bufs=1))
        qpool = ctx.enter_context(tc.tile_pool(name="qpool", bufs=2))
        xpool = ctx.enter_context(tc.tile_pool(name="xpool", bufs=3))
        cspool = ctx.enter_context(tc.tile_pool(name="cspool", bufs=2))
        rtmp = ctx.enter_context(tc.tile_pool(name="rtmp", bufs=2))
        mpool = ctx.enter_context(tc.tile_pool(name="mpool", bufs=2))
        ppool = ctx.enter_context(tc.tile_pool(name="ppool", bufs=2))
        ptpool = ctx.enter_context(tc.tile_pool(name="ptpool", bufs=2))
        ypool = ctx.enter_context(tc.tile_pool(name="ypool", bufs=2))
        lpool = ctx.enter_context(tc.tile_pool(name="lpool", bufs=2))
        psum = ctx.enter_context(tc.tile_pool(name="psum", bufs=2, space="PSUM"))
        psum1 = ctx.enter_context(tc.tile_pool(name="psum1", bufs=1, space="PSUM"))

        # constants
        ident = const.tile([128, 128], dt.bfloat16, tag="ident", name="ident")
        make_identity(nc, ident)
        wq_sb, wk_sb, wv_sb = [], [], []
        for c in range(8):
            for name, dram, lst in (
                ("wq", w2q, wq_sb),
                ("wk", w2k, wk_sb),
                ("wv", w2v, wv_sb),
            ):
                wt = const.tile([128, 128], dt.bfloat16, tag=f"{name}{c}", name=f"{name}{c}")
                nc.sync.dma_start(wt[:], dram[c])
                lst.append(wt)

        # resident K^T (per chunk c and 512-key stripe s) and V (per-stripe)
        KT = {}
        for s in range(n_kv_stripes):
            for c in range(8):
                KT[(c, s)] = kv.tile([128, 512], dt.bfloat16, tag=f"kt{c}_{s}",
                                     name=f"kt{c}_{s}")
        V = [
            kv.tile([128, 4, 1024], dt.bfloat16, tag=f"v{s}", name=f"v{s}")
            for s in range(n_kv_stripes)
        ]
        # Q^T streamed per 512-token stripe: tags per chunk
        QT = {}

        def proj_rope_pair(cp, sl, x_dram, w_sb, cos_dram, sin_dram, out_e, out_o,
                           sl_out, do_v, v_stripe):
            """One head-pair (chunks cp, cp+4) over one 512-token stripe.

            sl: slice in the source token axis; sl_out: slice in out tiles.
            """
            xa = xpool.tile([128, 512], dt.bfloat16, tag="xa", name="xa")
            xb = xpool.tile([128, 512], dt.bfloat16, tag="xb", name="xb")
            nc.sync.dma_start(xa[:], x_dram[cp, :, sl])
            nc.sync.dma_start(xb[:], x_dram[cp + 4, :, sl])
            cos = cspool.tile([128, 512], dt.bfloat16, tag=f"cos{cp}", name=f"cos{cp}")
            sin = cspool.tile([128, 512], dt.bfloat16, tag=f"sin{cp}", name=f"sin{cp}")
            nc.sync.dma_start(cos[:], cos_dram[cp, :, sl])
            nc.sync.dma_start(sin[:], sin_dram[cp, :, sl])

            pe = psum.tile([128, 512], dt.float32, tag="A", name="A")
            po = psum.tile([128, 512], dt.float32, tag="B", name="B")
            nc.tensor.matmul(pe[:], lhsT=w_sb[cp][:], rhs=xa[:], start=True, stop=True)
            nc.tensor.matmul(po[:], lhsT=w_sb[cp + 4][:], rhs=xb[:], start=True,
                             stop=True)
            ke = rtmp.tile([128, 512], dt.bfloat16, tag="ke", name="ke")
            ko = rtmp.tile([128, 512], dt.bfloat16, tag="ko", name="ko")
            nc.scalar.copy(ke[:], pe[:])
            nc.scalar.copy(ko[:], po[:])
            ta = rtmp.tile([128, 512], dt.bfloat16, tag="ta", name="ta")
            tb = rtmp.tile([128, 512], dt.bfloat16, tag="tb", name="tb")
            # out_e = ke*cos - ko*sin ; out_o = ke*sin + ko*cos  (all DVE:
            # gpsimd TT is ~3x slower and cross-engine hops serialize)
            nc.vector.tensor_mul(ta[:], ke[:], cos[:])
            nc.vector.tensor_mul(tb[:], ko[:], sin[:])
            nc.vector.tensor_sub(out_e[:, sl_out], ta[:], tb[:])
            ta2 = rtmp.tile([128, 512], dt.bfloat16, tag="ta", name="ta")
            tb2 = rtmp.tile([128, 512], dt.bfloat16, tag="tb", name="tb")
            nc.vector.tensor_mul(ta2[:], ke[:], sin[:])
            nc.vector.tensor_mul(tb2[:], ko[:], cos[:])
            nc.vector.tensor_add(out_o[:, sl_out], ta2[:], tb2[:])

            if do_v:
                va = psum1.tile([128, 4, 128], dt.float32, tag="VA", name="VA")
                vb = psum1.tile([128, 4, 128], dt.float32, tag="VB", name="VB")
                for sub in range(4):
                    nc.tensor.matmul(
                        va[:, sub, :], lhsT=xa[:, ts(sub, 128)], rhs=wv_sb[cp][:],
                        start=True, stop=True,
                    )
                    nc.tensor.matmul(
                        vb[:, sub, :], lhsT=xb[:, ts(sub, 128)], rhs=wv_sb[cp + 4][:],
                        start=True, stop=True,
                    )
                nc.any.tensor_copy(v_stripe[:, :, ds(cp * 128, 128)], va[:])
                nc.any.tensor_copy(v_stripe[:, :, ds((cp + 4) * 128, 128)], vb[:])

        def emit_kv_stripe(s):
            sl = ds(s * 512, 512)
            for cp in range(4):
                proj_rope_pair(cp, sl, xpT, wk_sb, cosk, sink,
                               KT[(cp, s)], KT[(cp + 4, s)], ds(0, 512), True, V[s])

        def emit_q_stripe(s):
            sl = ds(s * 512, 512)
            for c in range(8):
                QT[(c, s)] = qpool.tile([128, 512], dt.bfloat16, tag=f"qt{c}",
                                        name=f"qt{c}")
            for cp in range(4):
                proj_rope_pair(cp, sl, xqT, wq_sb, cosq, sinq,
                               QT[(cp, s)], QT[(cp + 4, s)], ds(0, 512), False, None)

        # ---- Phases B+C interleaved: Q^T stripe then its 4 q-tiles ----
        def emit_q_tile(j):
            nst = _n_stripes(j)
            nblk = 2 * j + 2
            y_lo = psum1.tile([128, 512], dt.float32, tag="YL", name="YL")
            y_hi = psum1.tile([128, 512], dt.float32, tag="YH", name="YH")
            l_parts = lpool.tile([128, 16], dt.float32, tag="lp", name="lp")
            qs, qoff = j // 4, (j % 4) * 128
            for s in range(nst):
                w = 512 if s < nst - 1 else _last_width(j)
                S = psum.tile([128, 512], dt.float32, tag="A", name="A")
                for c in range(8):
                    nc.tensor.matmul(
                        S[:, :w],
                        lhsT=QT[(c, qs)][:, ds(qoff, 128)],
                        rhs=KT[(c, s)][:, :w],
                        start=(c == 0), stop=(c == 7),
                    )
                if s == nst - 1:
                    mk = mpool.tile([128, 512], dt.float32, tag="mask", name="mask")
                    nc.sync.dma_start(mk[:], masks[j])
                    nc.vector.tensor_add(S[:, :w], S[:, :w], mk[:, :w])
                P = ppool.tile([128, 512], dt.bfloat16, tag="p", name="p")
                nc.scalar.activation(
                    P[:, :w], S[:, :w], mybir.ActivationFunctionType.Exp,
                    scale=GAMMA, accum_out=l_parts[:, ds(s, 1)],
                )
                nb = w // 128
                pt_ps = psum.tile([128, 512], dt.bfloat16, tag="B", name="B")
                for b in range(nb):
                    nc.tensor.transpose(pt_ps[:, ts(b, 128)], P[:, ts(b, 128)],
                                        ident[:])
                pt = ptpool.tile([128, 512], dt.bfloat16, tag="pt", name="pt")
                nc.scalar.copy(pt[:, :w], pt_ps[:, :w])
                for b in range(nb):
                    blk = s * 4 + b
                    vs = V[blk // 4]
                    nc.tensor.matmul(y_lo[:], lhsT=pt[:, ts(b, 128)],
                                     rhs=vs[:, blk % 4, 0:512],
                                     start=(blk == 0), stop=(blk == nblk - 1))
                    nc.tensor.matmul(y_hi[:], lhsT=pt[:, ts(b, 128)],
                                     rhs=vs[:, blk % 4, 512:1024],
                                     start=(blk == 0), stop=(blk == nblk - 1))
            lsum = lpool.tile([128, 1], dt.float32, tag="ls", name="ls")
            linv = lpool.tile([128, 1], dt.float32, tag="li", name="li")
            nc.vector.tensor_reduce(lsum[:], l_parts[:, :nst],
                                    mybir.AxisListType.X, mybir.AluOpType.add)
            nc.vector.reciprocal(linv[:], lsum[:])
            y_sb = ypool.tile([128, 1024], dt.float32, tag="y", name="y")
            nc.vector.tensor_scalar_mul(y_sb[:, 0:512], y_lo[:], linv[:])
            nc.vector.tensor_scalar_mul(y_sb[:, 512:1024], y_hi[:], linv[:])
            nc.sync.dma_start(y[ts(j, 128), :], y_sb[:])

        # Interleave: KV-stripe production feeds the two q-tiles that first
        # need it; Q-stripe production just before its first consumer.
        emit_q_stripe(0)
        for s in range(n_kv_stripes):
            emit_kv_stripe(s)
            for j in (2 * s, 2 * s + 1):
                if j >= NQ:
                    continue
                if j % 4 == 0 and j // 4 > 0:
                    emit_q_stripe(j // 4)
                emit_q_tile(j)

    nc.compile()
    return nc


# ------------------------- host side -------------------------


def prep_core_inputs(xb, w2q, w2k, w2v, cos_t, sin_t, parity, NQ, T):
    """Inputs for one core: batch slice xb (T, 1024) fp32, parity 0/1."""
    q_tiles = [2 * j + parity for j in range(NQ)]
    xpT = np.ascontiguousarray(xb.T[PERM].reshape(8, 128, T)).astype(BF16)
    cols = np.concatenate([np.arange(G * 128, (G + 1) * 128) for G in q_tiles])
    xqT = np.ascontiguousarray(xpT[:, :, cols])
    cosq = np.ascontiguousarray(cos_t[:, :, cols])
    sinq = np.ascontiguousarray(sin_t[:, :, cols])
    return {
        "xpT": xpT,
        "xqT": xqT,
        "w2q": w2q,
        "w2k": w2k,
        "w2v": w2v,
        "cosk": cos_t,
        "sink": sin_t,
        "cosq": cosq,
        "sinq": sinq,
        "masks": _masks_for_core(q_tiles, NQ),
    }


def core_model(inp, NQ):
    """Numpy model of what one core's program computes (fp32 math, for tests)."""
    T = inp["xpT"].shape[2]
    xpT = inp["xpT"].astype(np.float32)
    xqT = inp["xqT"].astype(np.float32)
    cosk = inp["cosk"].astype(np.float32)
    sink = inp["sink"].astype(np.float32)
    cosq = inp["cosq"].astype(np.float32)
    sinq = inp["sinq"].astype(np.float32)
    w2q = inp["w2q"].astype(np.float32)
    w2k = inp["w2k"].astype(np.float32)
    w2v = inp["w2v"].astype(np.float32)

    def proj_T(xT, w2):  # -> [8, 128, n]
        return np.stack([w2[c].T @ xT[c] for c in range(8)])

    def rope(zT, cos, sin):
        out = np.empty_like(zT)
        for c in range(4):
            e, o = zT[c], zT[c + 4]
            out[c] = e * cos[c] - o * sin[c]
            out[c + 4] = e * sin[c] + o * cos[c]
        return out

    kT = rope(proj_T(xpT, w2k), cosk, sink).reshape(1024, T)
    qT = rope(proj_T(xqT, w2q), cosq, sinq).reshape(1024, NQ * 128)
    v = np.concatenate([w2v[c].T @ xpT[c] for c in range(8)], axis=0).T  # [T, 1024]

    y = np.zeros((NQ * 128, 1024), dtype=np.float32)
    for j in range(NQ):
        nblk = 2 * j + 2
        q = qT[:, j * 128:(j + 1) * 128].T  # [128, 1024]
        keys = kT[:, : nblk * 128]
        S = q @ keys  # [128, nblk*128]
        mfull = inp["masks"][j]
        k0 = 4 * (_n_stripes(j) - 1) * 128
        S[:, k0:] += mfull[:, : nblk * 128 - k0]
        P = np.exp(GAMMA * S)
        y[j * 128:(j + 1) * 128] = (P @ v[: nblk * 128]) / P.sum(1, keepdims=True)
    return y


_NC_CACHE = {}
last_in_maps = None
last_nc = None


def kernel(x, w_q, w_k, w_v):
    global last_in_maps, last_nc
    from concourse.bass_utils import run_bass_kernel_spmd

    B, T, D = x.shape
    assert (B, T, D) == (4, 4096, 1024)
    NQ = 16
    x = np.asarray(x, dtype=np.float32)
    w2q = _block_weights(np.asarray(w_q, dtype=np.float32))
    w2k = _block_weights(np.asarray(w_k, dtype=np.float32))
    w2v = _block_weights(np.asarray(w_v, dtype=np.float32))
    cos_t, sin_t = _rope_tables(T)

    in_maps = []
    for core in range(8):
        b, parity = core // 2, core % 2
        in_maps.append(
            prep_core_inputs(x[b], w2q, w2k, w2v, cos_t, sin_t, parity, NQ, T)
        )
    last_in_maps = in_maps

    key = (T, NQ)
    if key not in _NC_CACHE:
        _NC_CACHE[key] = build_nc(T, NQ)
    nc = _NC_CACHE[key]
    last_nc = nc

    res = run_bass_kernel_spmd(nc, in_maps, core_ids=list(range(8)))
    out = np.zeros((B, T, D), dtype=np.float32)
    for core in range(8):
        b, parity = core // 2, core % 2
        yk = res.results[core]["y"].reshape(NQ, 128, D)
        for j in range(NQ):
            G = 2 * j + parity
            out[b, G * 128:(G + 1) * 128, :] = yk[j][:, INV_PERM]
    return out



# revision 8
# speedup vs baseline: 1.0330x; 1.0330x over previous
"""Trainium2 Bass kernel for nn_MultiHeadAttention_85229331022244.

Computation (per batch b):
  xh = x.reshape(B,T,64,16); q/k/v = per-head 64x64 projections of xh
  q,k: interleaved RoPE over the FULL 1024-dim feature axis
  scores = q @ k.T / sqrt(1024)  (single attention map over full D)
  causal softmax; y = attn @ v

Sharding: core i -> batch i//2, q-block parity i%2 (even/odd 128-row q-blocks
interleaved between the two cores of a batch).  Every core runs an identical
program; parity differences are carried purely in DATA (a per-core key-block
permutation + 6 multiplicative mask tiles).

Dataflow (S-transposed flash):
  - heads reordered even-first and paired so projections are 8 block-diagonal
    128x128 matmuls producing K^T/Q^T in [feature, token] layout (as baseline).
  - scores computed TRANSPOSED: S^T[key, q] tiles [128, 256], so exp(S^T) is
    directly the lhsT of the attn@V matmuls -- no P transposes.
  - softmax row sums via N=1 matmuls against a ones vector.
  - causal masking via 6 multiplicative [128,128] masks (per-core data).
  - Q projection inputs are column slots of the SAME x stripes used for K/V
    (per-core key permutation puts each core's q-blocks at slots 1,3).
"""

import math
from contextlib import ExitStack

import numpy as np
import ml_dtypes

import concourse.bass as bass
import concourse.mybir as mybir
import concourse.tile as tile
from concourse import bacc
from concourse.bass import ts, ds

BF16 = ml_dtypes.bfloat16

D_MODEL = 1024
N_HEADS = 16
HEAD_D = 64
ROPE_BASE = 10000.0
GAMMA = 1.0 / math.sqrt(D_MODEL)
T = 4096
NSTR = T // 512  # 8 key stripes / q groups per core

HEAD_PAIRS = [(0, 2), (4, 6), (8, 10), (12, 14), (1, 3), (5, 7), (9, 11), (13, 15)]


def _feature_perm():
    perm = np.zeros(1024, dtype=np.int64)
    for c, (ha, hb) in enumerate(HEAD_PAIRS):
        for p in range(128):
            h = ha if p < 64 else hb
            perm[c * 128 + p] = (p % 64) * 16 + h
    return perm


PERM = _feature_perm()
INV_PERM = np.argsort(PERM)


def _block_weights(w):
    out = np.zeros((8, 128, 128), dtype=np.float32)
    for c, (ha, hb) in enumerate(HEAD_PAIRS):
        out[c, :64, :64] = w[:, :, ha]
        out[c, 64:, 64:] = w[:, :, hb]
    return out.astype(BF16)


def _freqs():
    p = np.arange(128)
    f = np.zeros((4, 128), dtype=np.float64)
    for c in range(4):
        fidx = (p % 64) * 8 + (2 * c + p // 64)
        f[c] = ROPE_BASE ** (-fidx / 512.0)
    return f


FREQS = _freqs()


def _kcols(parity):
    order = []
    for s in range(NSTR):
        if parity == 0:
            order += [4 * s + 1, 4 * s + 0, 4 * s + 3, 4 * s + 2]
        else:
            order += [4 * s + 0, 4 * s + 1, 4 * s + 2, 4 * s + 3]
    return np.concatenate([np.arange(128) + 128 * b for b in order])


def _msel(parity):
    r = np.arange(128)[:, None]
    c = np.arange(128)[None, :]
    tri = (r <= c).astype(np.float32)
    ones = np.ones((128, 128), np.float32)
    zeros = np.zeros((128, 128), np.float32)
    if parity == 0:
        m = [zeros, tri, zeros, zeros, zeros, tri]
    else:
        m = [ones, tri, zeros, zeros, ones, tri]
    return np.stack(m).astype(BF16)


def _rope_tables_neg(kc):
    """(-cos, -sin) tables at global t columns kc: each [4, 128, T] fp32."""
    t = np.asarray(kc, dtype=np.float64)
    ang = FREQS[:, :, None] * t[None, None, :]
    return -np.cos(ang), -np.sin(ang)


# ------------------------- device program -------------------------


def build_nc():
    dt = mybir.dt
    nc = bacc.Bacc("TRN2", target_bir_lowering=False)
    xS = nc.dram_tensor("xS", [NSTR, 128, 8, 512], dt.bfloat16, kind="ExternalInput")
    csS = nc.dram_tensor("csS", [NSTR, 128, 4, 2, 512], dt.bfloat16,
                         kind="ExternalInput")
    w2q = nc.dram_tensor("w2q", [8, 128, 128], dt.bfloat16, kind="ExternalInput")
    w2k = nc.dram_tensor("w2k", [8, 128, 128], dt.bfloat16, kind="ExternalInput")
    w2v = nc.dram_tensor("w2v", [8, 128, 128], dt.bfloat16, kind="ExternalInput")
    mselD = nc.dram_tensor("msel", [6, 128, 128], dt.bfloat16, kind="ExternalInput")
    y = nc.dram_tensor("y", [2 * NSTR, 128, 1024], dt.bfloat16,
                       kind="ExternalOutput")

    with tile.TileContext(nc) as tc, ExitStack() as ctx:
        const = ctx.enter_context(tc.tile_pool(name="const", bufs=1))
        kv = ctx.enter_context(tc.tile_pool(name="kv", bufs=1))
        xpool = ctx.enter_context(tc.tile_pool(name="xpool", bufs=2))
        cspool = ctx.enter_context(tc.tile_pool(name="cspool", bufs=1))
        qpool = ctx.enter_context(tc.tile_pool(name="qpool", bufs=2))
        qcs = ctx.enter_context(tc.tile_pool(name="qcs", bufs=2))
        rtmp = ctx.enter_context(tc.tile_pool(name="rtmp", bufs=2))
        ptpool = ctx.enter_context(tc.tile_pool(name="ptpool", bufs=1))
        ypool = ctx.enter_context(tc.tile_pool(name="ypool", bufs=2))
        lpool = ctx.enter_context(tc.tile_pool(name="lpool", bufs=2))
        psS = ctx.enter_context(tc.tile_pool(name="psS", bufs=2, space="PSUM"))
        psY = ctx.enter_context(tc.tile_pool(name="psY", bufs=1, space="PSUM"))
        psL = ctx.enter_context(tc.tile_pool(name="psL", bufs=1, space="PSUM"))
        psP = ctx.enter_context(tc.tile_pool(name="psP", bufs=1, space="PSUM"))
        psV = ctx.enter_context(tc.tile_pool(name="psV", bufs=1, space="PSUM"))

        # ---- constants ----
        ones = const.tile([128, 1], dt.bfloat16, tag="ones", name="ones")
        nc.gpsimd.memset(ones[:], 1.0)
        wq_sb, wk_sb, wv_sb = [], [], []
        for c in range(8):
            for nm, dram, lst in (("wq", w2q, wq_sb), ("wk", w2k, wk_sb),
                                  ("wv", w2v, wv_sb)):
                wt = const.tile([128, 128], dt.bfloat16, tag=f"{nm}{c}",
                                name=f"{nm}{c}")
                nc.sync.dma_start(wt[:], dram[c])
                lst.append(wt)
        msel = []
        for i in range(6):
            mt = const.tile([128, 128], dt.bfloat16, tag=f"msel{i}",
                            name=f"msel{i}")
            nc.sync.dma_start(mt[:], mselD[i])
            msel.append(mt)

        # resident K^T per (chunk, stripe) and V per local key block
        KT = {}
        for s in range(NSTR):
            for c in range(8):
                KT[(c, s)] = kv.tile([128, 512], dt.bfloat16, tag=f"kt{c}_{s}",
                                     name=f"kt{c}_{s}")
        V = [kv.tile([128, 1024], dt.bfloat16, tag=f"v{kb}", name=f"v{kb}")
             for kb in range(4 * NSTR)]

        def rope6(out_e, out_o, ke, ko, cos, sin, w):
            """out_e = ke*cos - ko*sin ; out_o = ke*sin + ko*cos (width w)."""
            ta = rtmp.tile([128, 512], dt.bfloat16, tag="ta", name="ta")
            tb = rtmp.tile([128, 512], dt.bfloat16, tag="tb", name="tb")
            nc.vector.tensor_mul(ta[:, :w], ke[:, :w], cos)
            nc.vector.tensor_mul(tb[:, :w], ko[:, :w], sin)
            nc.vector.tensor_sub(out_e, ta[:, :w], tb[:, :w])
            ta2 = rtmp.tile([128, 512], dt.bfloat16, tag="ta", name="ta")
            tb2 = rtmp.tile([128, 512], dt.bfloat16, tag="tb", name="tb")
            nc.vector.tensor_mul(ta2[:, :w], ke[:, :w], sin)
            nc.vector.tensor_mul(tb2[:, :w], ko[:, :w], cos)
            nc.vector.tensor_add(out_o, ta2[:, :w], tb2[:, :w])

        QT = {}

        def emit_stripe(s):
            xt = xpool.tile([128, 8, 512], dt.bfloat16, tag="xt", name="xt")
            nc.sync.dma_start(xt[:], xS[s])
            cs = cspool.tile([128, 4, 2, 512], dt.bfloat16, tag="cs", name="cs")
            nc.sync.dma_start(cs[:], csS[s])

            # K projection + RoPE (feature-on-partition layout)
            for cp in range(4):
                pe = psP.tile([128, 512], dt.float32, tag="pe", name="pe")
                po = psP.tile([128, 512], dt.float32, tag="po", name="po")
                nc.tensor.matmul(pe[:], lhsT=wk_sb[cp][:], rhs=xt[:, cp, :],
                                 start=True, stop=True)
                nc.tensor.matmul(po[:], lhsT=wk_sb[cp + 4][:],
                                 rhs=xt[:, cp + 4, :], start=True, stop=True)
                ke = rtmp.tile([128, 512], dt.bfloat16, tag="ke", name="ke")
                ko = rtmp.tile([128, 512], dt.bfloat16, tag="ko", name="ko")
                nc.scalar.copy(ke[:], pe[:])
                nc.scalar.copy(ko[:], po[:])
                rope6(KT[(cp, s)][:], KT[(cp + 4, s)][:], ke, ko,
                      cs[:, cp, 0, :], cs[:, cp, 1, :], 512)

            # V projection per local key block (keys on partitions)
            for j in range(4):
                kb = 4 * s + j
                for half in range(2):
                    pv = psV.tile([128, 512], dt.float32, tag="pv", name="pv")
                    for cc in range(4):
                        c = 4 * half + cc
                        nc.tensor.matmul(
                            pv[:, ts(cc, 128)],
                            lhsT=xt[:, c, ds(128 * j, 128)], rhs=wv_sb[c][:],
                            start=True, stop=True)
                    if half == 0:
                        nc.scalar.copy(V[kb][:, ds(0, 512)], pv[:])
                    else:
                        nc.vector.tensor_copy(V[kb][:, ds(512, 512)], pv[:])

            # Q tables (columns at slots 1,3 of this stripe) + Q proj + RoPE
            for cp in range(4):
                qc = qcs.tile([128, 256], dt.bfloat16, tag=f"qc{cp}",
                              name=f"qc{cp}")
                qs = qcs.tile([128, 256], dt.bfloat16, tag=f"qs{cp}",
                              name=f"qs{cp}")
                nc.vector.tensor_copy(qc[:, 0:128], cs[:, cp, 0, ds(128, 128)])
                nc.vector.tensor_copy(qc[:, 128:256], cs[:, cp, 0, ds(384, 128)])
                nc.vector.tensor_copy(qs[:, 0:128], cs[:, cp, 1, ds(128, 128)])
                nc.vector.tensor_copy(qs[:, 128:256], cs[:, cp, 1, ds(384, 128)])

                pe = psP.tile([128, 512], dt.float32, tag="pe", name="pe")
                po = psP.tile([128, 512], dt.float32, tag="po", name="po")
                for sl, off in ((0, 128), (1, 384)):
                    nc.tensor.matmul(pe[:, ts(sl, 128)], lhsT=wq_sb[cp][:],
                                     rhs=xt[:, cp, ds(off, 128)],
                                     start=True, stop=True)
                    nc.tensor.matmul(po[:, ts(sl, 128)], lhsT=wq_sb[cp + 4][:],
                                     rhs=xt[:, cp + 4, ds(off, 128)],
                                     start=True, stop=True)
                ke = rtmp.tile([128, 512], dt.bfloat16, tag="ke", name="ke")
                ko = rtmp.tile([128, 512], dt.bfloat16, tag="ko", name="ko")
                nc.scalar.copy(ke[:, :256], pe[:, :256])
                nc.scalar.copy(ko[:, :256], po[:, :256])
                qte = qpool.tile([128, 256], dt.bfloat16, tag=f"qt{cp}",
                                 name=f"qt{cp}")
                qto = qpool.tile([128, 256], dt.bfloat16, tag=f"qt{cp + 4}",
                                 name=f"qt{cp + 4}")
                rope6(qte[:], qto[:], ke, ko, qc[:], qs[:], 256)
                QT[cp] = qte
                QT[cp + 4] = qto

        def emit_q_group(g):
            nkb = 4 * g + 4
            pts = []
            for kb in range(nkb):
                S = psS.tile([128, 256], dt.float32, tag="S", name="S")
                for c in range(8):
                    nc.tensor.matmul(
                        S[:],
                        lhsT=KT[(c, kb // 4)][:, ts(kb % 4, 128)],
                        rhs=QT[c][:],
                        start=(c == 0), stop=(c == 7))
                pt = ptpool.tile([128, 256], dt.bfloat16, tag=f"pt{kb}",
                                 name=f"pt{kb}")
                nc.scalar.activation(pt[:], S[:],
                                     mybir.ActivationFunctionType.Exp,
                                     scale=GAMMA)
                pts.append(pt)
            # causal masking multiplies (last stripe's 4 blocks)
            for j in range(4):
                kb = 4 * g + j
                nc.vector.tensor_mul(pts[kb][:, 0:128], pts[kb][:, 0:128],
                                     msel[j][:])
            for jj, j in enumerate((2, 3)):
                kb = 4 * g + j
                nc.vector.tensor_mul(pts[kb][:, 128:256], pts[kb][:, 128:256],
                                     msel[4 + jj][:])
            # attn @ V + row sums, per owned q block m
            L = psL.tile([128, 2], dt.float32, tag="L", name="L")
            for m in range(2):
                Y = psY.tile([128, 1024], dt.float32, tag="Y", name="Y")
                for kb in range(nkb):
                    lhs = pts[kb][:, ds(128 * m, 128)]
                    nc.tensor.matmul(Y[:, 0:512], lhsT=lhs, rhs=V[kb][:, 0:512],
                                     start=(kb == 0), stop=(kb == nkb - 1))
                    nc.tensor.matmul(Y[:, 512:1024], lhsT=lhs,
                                     rhs=V[kb][:, 512:1024],
                                     start=(kb == 0), stop=(kb == nkb - 1))
                    nc.tensor.matmul(L[:, ds(m, 1)], lhsT=lhs, rhs=ones[:],
                                     start=(kb == 0), stop=(kb == nkb - 1))
                linv = lpool.tile([128, 1], dt.float32, tag="li", name="li")
                nc.vector.reciprocal(linv[:], L[:, ds(m, 1)])
                y_sb = ypool.tile([128, 1024], dt.bfloat16, tag="y", name="y")
                nc.vector.tensor_scalar_mul(y_sb[:], Y[:], linv[:])
                nc.sync.dma_start(y[2 * g + m], y_sb[:])

        for s in range(NSTR):
            emit_stripe(s)
            emit_q_group(s)

    nc.compile()
    return nc


# ------------------------- host side -------------------------


def prep_core_inputs(xb, w2q, w2k, w2v, cos4, sin4, parity):
    """Inputs for one core: batch slice xb (T, 1024) fp32, parity 0/1.

    cos4/sin4: negated tables [4, 128, T] fp32 at natural t (pre-permutation).
    """
    kc = _kcols(parity)
    xpT = np.ascontiguousarray(xb.T[PERM]).reshape(8, 128, T)
    xperm = xpT[:, :, kc]
    xS = np.ascontiguousarray(
        xperm.reshape(8, 128, NSTR, 512).transpose(2, 1, 0, 3)).astype(BF16)
    cs = np.stack([cos4[:, :, kc], sin4[:, :, kc]], axis=2)  # [4,128,2,T]
    csS = np.ascontiguousarray(
        cs.reshape(4, 128, 2, NSTR, 512).transpose(3, 1, 0, 2, 4)).astype(BF16)
    return {
        "xS": xS,
        "csS": csS,
        "w2q": w2q,
        "w2k": w2k,
        "w2v": w2v,
        "msel": _msel(parity),
    }


_NC_CACHE = {}
last_in_maps = None
last_nc = None


def kernel(x, w_q, w_k, w_v):
    global last_in_maps, last_nc
    from concourse.bass_utils import run_bass_kernel_spmd

    B, Tx, D = x.shape
    assert (B, Tx, D) == (4, 4096, 1024)
    x = np.asarray(x, dtype=np.float32)
    w2q = _block_weights(np.asarray(w_q, dtype=np.float32))
    w2k = _block_weights(np.asarray(w_k, dtype=np.float32))
    w2v = _block_weights(np.asarray(w_v, dtype=np.float32))
    cos4, sin4 = _rope_tables_neg(np.arange(T))

    in_maps = []
    for core in range(8):
        b, parity = core // 2, core % 2
        in_maps.append(
            prep_core_inputs(x[b], w2q, w2k, w2v, cos4, sin4, parity))
    last_in_maps = in_maps

    if "nc" not in _NC_CACHE:
        _NC_CACHE["nc"] = build_nc()
    nc = _NC_CACHE["nc"]
    last_nc = nc

    res = run_bass_kernel_spmd(nc, in_maps, core_ids=list(range(8)))
    out = np.zeros((B, Tx, D), dtype=np.float32)
    for core in range(8):
        b, parity = core // 2, core % 2
        yk = res.results[core]["y"].astype(np.float32)  # [16, 128, 1024]
        for g in range(NSTR):
            for m in range(2):
                G = 4 * g + 2 * m + parity
                out[b, 128 * G:128 * (G + 1), :] = yk[2 * g + m][:, INV_PERM]
    return out
